# revision 1
# baseline (speedup 1.0000x reference)
"""Trainium2 Bass kernel v2 for a 2-layer GCN + global mean pool + sigmoid.

Reference math:
    h1 = relu(scatter_add_dst(x[src]) @ W1)       # = relu((A @ x) @ W1)
    g  = mean_pool(scatter_add_dst((h1 W2)[src]), batch)
    out = sigmoid(g @ Wout + bout)

Distribution (8 cores): nodes in 8 contiguous blocks of NPC = N/8; layer-1
edges sharded by DST block, layer-2 collapsed into a per-core count-matrix
matmul (K^T @ h1) sharded by SRC block, AllReduced (tiny [G,D]).

v2 speedups over the per-tile indirect-DMA baseline:
  * Edge gathers use bulk `dma_gather` (one instruction per ~10k edges instead
    of one indirect DMA per 128 edges) - SWDGE descriptor generation drops
    from ~1.6ms to ~80us. int16 gather indices force splitting x into
    source-chunks of 32768 rows; per-(window,chunk) segments are padded to
    128-slot tiles, uniformly across cores (max), so one SPMD program fits
    all 8 cores.
  * Messages, selection matrices, and weights are bf16: 1 cycle/row PE
    matmuls (fp32 is 4) and half the SBUF footprint.
  * Windows are processed in PSUM-resident groups of NW=7, one PSUM bank
    per window (matmul start=True pending-zeroes a whole 2KB bank, so
    accumulators cannot share banks); the 4 chunk gathers of a group
    accumulate straight into PSUM, and selection matmuls are round-robined
    across the 7 banks (same-bank accumulation chains serialize at ~324ns
    per matmul on HW, alternating banks pipeline).
  * The aggregation is computed transposed (aggT = msgs^T @ S per tile), so
    h1 = relu(aggT^T @ W1) needs no per-window PE transpose.
"""

import sys

sys.path.insert(0, "/opt/trn_rl_repo")

import numpy as np
import ml_dtypes

bfloat16 = ml_dtypes.bfloat16

P = 128
CHUNK = 32768  # dma_gather idx is int16: source chunks of <=32768 rows
NW = 7         # windows per PSUM-resident group (98 = 14 * 7); one PSUM bank each

FULL_N = 100000
FULL_D = 128
FULL_G = 256
FULL_CORES = 8

DEBUG_DUMP = False  # adds per-window aggT/h1 debug outputs to the program
SKIP_GATHER = False  # timing probe: elide dma_gathers (msgs left stale)
GATHER_ONLY = False  # timing probe: elide sT/matmul/post-proc consumers


def _ceil(a, b):
    return -(-a // b)


# --------------------------------------------------------------------------
# host-side preprocessing
# --------------------------------------------------------------------------
def host_prep(x, edge_index, batch, n_cores, n_graphs):
    N, D = x.shape
    assert N % n_cores == 0
    NPC = N // n_cores
    W = _ceil(NPC, P)
    n_chunks = _ceil(N, CHUNK)
    n_groups = _ceil(W, NW)
    G = n_graphs

    src = np.ascontiguousarray(edge_index[0]).astype(np.int64)
    dst = np.ascontiguousarray(edge_index[1]).astype(np.int64)
    b = np.asarray(batch).astype(np.int64)
    g_of_dst = b[dst]

    cnt = np.bincount(b, minlength=G).astype(np.float64)
    recip = (1.0 / np.maximum(cnt, 1.0)).astype(np.float32)

    core1 = dst // NPC
    core2 = src // NPC

    # per-core edge arrays sorted by (group, chunk, window) + (w,c) counts
    percore = []
    ec_all = np.zeros((n_cores, W, n_chunks), np.int64)
    for i in range(n_cores):
        m = core1 == i
        es = src[m]
        dl = dst[m] - i * NPC
        w = dl >> 7
        c = es // CHUNK
        order = np.lexsort((w, c, w // NW))
        es, dl, w, c = es[order], dl[order], w[order], c[order]
        ec = np.bincount(w * n_chunks + c, minlength=W * n_chunks)
        ec_all[i] = ec.reshape(W, n_chunks)
        percore.append((es, dl, w, c))

    # uniform tile counts per (window, chunk): max over cores
    T = (ec_all.max(axis=0) + P - 1) // P  # [W, n_chunks]
    empty = T.sum(axis=1) == 0
    T[empty, 0] = 1  # every window needs >=1 tile so PSUM gets initialized

    # global tile layout in (group, chunk, window) order
    seg_tilebase = np.zeros((W, n_chunks), np.int64)
    cfirst = np.full(W, 0, np.int64)
    clast = np.full(W, 0, np.int64)
    for w in range(W):
        nz = np.flatnonzero(T[w] > 0)
        cfirst[w], clast[w] = nz[0], nz[-1]
    gathers = []
    cursor = 0
    for g in range(n_groups):
        ws = list(range(g * NW, min((g + 1) * NW, W)))
        for c in range(n_chunks):
            base = cursor
            segs = []
            for w in ws:
                t = int(T[w, c])
                if t == 0:
                    continue
                seg_tilebase[w, c] = cursor
                segs.append((w, t, c == cfirst[w], c == clast[w]))
                cursor += t
            nt = cursor - base
            if nt:
                rows = min(CHUNK, N - c * CHUNK)
                gathers.append(
                    dict(g=g, c=c, tile_base=base, ntiles=nt, rows=rows, segs=segs)
                )
    TT = cursor
    S = TT * P
    MAXTC = max(gh["ntiles"] for gh in gathers)

    cores = []
    for i in range(n_cores):
        es, dl, w, c = percore[i]
        key = w * n_chunks + c
        n = len(key)
        if n:
            change = np.empty(n, bool)
            change[0] = True
            change[1:] = key[1:] != key[:-1]
            run_start = np.flatnonzero(change)
            run_len = np.diff(np.append(run_start, n))
            rank = np.arange(n) - np.repeat(run_start, run_len)
            slot = seg_tilebase[w, c] * P + rank
        else:
            slot = np.zeros(0, np.int64)
        idx_stream = np.zeros(S, np.int16)
        dst_stream = np.full(S, -1.0, np.float32)
        idx_stream[slot] = (es - c * CHUNK).astype(np.int16)
        dst_stream[slot] = (dl - (w << 7)).astype(np.float32)
        # idx k of the stream lives at [partition k%16, col k//16], x8 replicas
        idx_all = np.tile(
            np.ascontiguousarray(idx_stream.reshape(S // 16, 16).T), (8, 1)
        )
        # slot = tile*128 + p  ->  dst_all[p, tile]
        dst_all = np.ascontiguousarray(dst_stream.reshape(TT, P).T).astype(bfloat16)

        m2 = core2 == i
        s2 = src[m2] - i * NPC
        g2 = g_of_dst[m2]
        KT = np.bincount(s2 * G + g2, minlength=W * P * G).astype(np.float32)
        KT = KT.reshape(W, P, G)
        kt_all = np.zeros((n_groups, P, NW * G), np.float32)
        for g in range(n_groups):
            for wi, w_ in enumerate(range(g * NW, min((g + 1) * NW, W))):
                kt_all[g, :, wi * G : (wi + 1) * G] = KT[w_]
        cores.append(
            dict(
                idx_all=idx_all,
                dst_all=dst_all,
                kt_all=kt_all.astype(bfloat16),
            )
        )

    struct = dict(
        W=W,
        NW=NW,
        n_groups=n_groups,
        n_chunks=n_chunks,
        TT=TT,
        S=S,
        MAXTC=MAXTC,
        gathers=gathers,
        NPC=NPC,
    )
    return dict(cores=cores, struct=struct, recip=recip)


# --------------------------------------------------------------------------
# bass program
# --------------------------------------------------------------------------
def build_bass(N, D, G, struct, n_cores, n_rep=1):
    import concourse.bass as bass  # noqa: F401
    import concourse.bacc as bacc
    import concourse.mybir as mybir
    from concourse.tile import TileContext

    f32 = mybir.dt.float32
    bf16 = mybir.dt.bfloat16
    i16 = mybir.dt.int16
    GT = _ceil(G, P)
    gp = [min(P, G - j * P) for j in range(GT)]

    W = struct["W"]
    n_groups = struct["n_groups"]
    TT = struct["TT"]
    S = struct["S"]
    MAXTC = struct["MAXTC"]
    gathers = struct["gathers"]

    nc = bacc.Bacc(trn_type="TRN2")

    xh_d = nc.declare_dram_parameter("xh", [N, D], bf16, isOutput=False)
    idx_d = nc.declare_dram_parameter("idx_all", [P, S // 16], i16, isOutput=False)
    dst_d = nc.declare_dram_parameter("dst_all", [P, TT], bf16, isOutput=False)
    kt_d = nc.declare_dram_parameter(
        "kt_all", [n_groups, P, NW * G], bf16, isOutput=False
    )
    w1_d = nc.declare_dram_parameter("W1h", [D, D], bf16, isOutput=False)
    io_d = nc.declare_dram_parameter("iota", [P, MAXTC * P], bf16, isOutput=False)
    w2_d = nc.declare_dram_parameter("W2", [D, D], f32, isOutput=False)
    wo_d = nc.declare_dram_parameter("Wout", [D, 1], f32, isOutput=False)
    rc_d = nc.declare_dram_parameter("recip", [P, GT], f32, isOutput=False)
    bo_d = nc.declare_dram_parameter("boutb", [P, 1], f32, isOutput=False)
    id_d = nc.declare_dram_parameter("ident32", [P, P], f32, isOutput=False)
    out_d = nc.declare_dram_parameter("out", [G, 1], f32, isOutput=True)

    cc_in = nc.dram_tensor("cc_in", [G, D], f32)
    cc_out = nc.dram_tensor(
        "cc_out", [G, D], f32, addr_space="Shared" if n_cores > 4 else "Local"
    )
    if DEBUG_DUMP:
        dbg_aggT = nc.declare_dram_parameter("dbg_aggT", [W, P, P], f32, isOutput=True)
        dbg_h1 = nc.declare_dram_parameter("dbg_h1", [W, P, P], f32, isOutput=True)

    # NOT the context-manager form: freeing the slot lets the final
    # TileContext's DMASW lane reuse it while it still holds cc_sem's value.
    cc_sem = nc.alloc_semaphore("cc_sem")
    for rep in range(n_rep):
        with TileContext(nc) as tc:
            with (
                tc.tile_pool(name="const", bufs=1) as cpool,
                tc.tile_pool(name="aggp", bufs=1, space="PSUM") as aggp,
                tc.tile_pool(name="spsum", bufs=1, space="PSUM") as spsum,
                tc.tile_pool(name="mpool", bufs=4) as mpool,
                tc.tile_pool(name="stpool", bufs=4) as stpool,
                tc.tile_pool(name="kpool", bufs=2) as kpool,
                tc.tile_pool(name="hpool", bufs=3) as hpool,
                tc.tile_pool(name="psum", bufs=2, space="PSUM") as psum,
            ):
                w1_sb = cpool.tile([D, D], bf16)
                nc.sync.dma_start(out=w1_sb[:], in_=w1_d[:, :])
                iota_sb = cpool.tile([P, MAXTC * P], bf16)
                nc.sync.dma_start(out=iota_sb[:], in_=io_d[:, :])
                idx_sb = cpool.tile([P, S // 16], i16)
                nc.sync.dma_start(out=idx_sb[:], in_=idx_d[:, :])
                dst_sb = cpool.tile([P, TT], bf16)
                nc.sync.dma_start(out=dst_sb[:], in_=dst_d[:, :])

                # touch on DVE once so later is_equal ops need only same-engine
                # ordering (TT codegen has few sync-wait slots)
                touch = cpool.tile([P, 2], bf16)
                nc.vector.tensor_copy(out=touch[:, 0:1], in_=dst_sb[:, 0:1])
                nc.vector.tensor_copy(out=touch[:, 1:2], in_=iota_sb[:, 0:1])

                s_all = spsum.tile([P, GT * D], f32, tag="s_all", name="s_all")

                for g in range(n_groups):
                    ws = list(range(g * NW, min((g + 1) * NW, W)))
                    # one PSUM bank per window: start=True pending-zeroes the whole
                    # 2KB zero region, so accumulators must not share banks.
                    # Fresh generation per group so the h1 reuse below alternates
                    # agg(g) -> h1(g) -> agg(g+1) in the pool's lifetime model.
                    agg_banks = [
                        aggp.tile([P, P], f32, tag=f"agg{b}", name=f"agg{b}_{g}")
                        for b in range(len(ws))
                    ]
                    kt_sb = kpool.tile([P, NW * G], bf16, tag="kt")
                    nc.sync.dma_start(out=kt_sb[:], in_=kt_d[g, :, :])

                    for gh in (gh for gh in gathers if gh["g"] == g):
                        c = gh["c"]
                        nt = gh["ntiles"]
                        tb = gh["tile_base"]
                        rows = gh["rows"]
                        msgs = mpool.tile([P, MAXTC * D], bf16, tag="msgs")
                        nc.gpsimd.dma_gather(
                            out_ap=msgs[:, : nt * D].rearrange("p (t e) -> p t e", e=D),
                            in_ap=xh_d[c * CHUNK : c * CHUNK + rows, :],
                            idxs_ap=idx_sb[:, tb * 8 : (tb + nt) * 8],
                            num_idxs=nt * P,
                            num_idxs_reg=nt * P,
                            elem_size=D,
                            # >1024 idxs with single_packet=True wedges the device
                            # (NRT_EXEC_UNIT_UNRECOVERABLE); per-desc packets work
                            single_packet=False,
                        )
                        sT = stpool.tile([P, MAXTC * P], bf16, tag="sT")
                        nc.vector.tensor_tensor(
                            out=sT[:, : nt * P].rearrange("p (t n) -> p t n", n=P),
                            in0=dst_sb[:, tb : tb + nt].to_broadcast([P, nt, P]),
                            in1=iota_sb[:, : nt * P].rearrange("p (t n) -> p t n", n=P),
                            op=mybir.AluOpType.is_equal,
                        )
                        # Round-robin the windows' tiles so consecutive matmuls
                        # hit different PSUM banks: same-bank accumulation
                        # chains serialize at ~324ns/matmul on HW, alternating
                        # banks pipeline at ~81ns.
                        toff = 0
                        segs_sched = []
                        for (w_, t_wc, first, last) in gh["segs"]:
                            segs_sched.append([w_ - g * NW, toff, t_wc, first, last])
                            toff += t_wc
                        for k in range(max(sg[2] for sg in segs_sched)):
                            for (wi, t0, t_wc, first, last) in segs_sched:
                                if k >= t_wc:
                                    continue
                                # aggT[feat, dst] += msgs_tile^T @ S_tile
                                nc.tensor.matmul(
                                    out=agg_banks[wi][:],
                                    lhsT=msgs[:, (t0 + k) * D : (t0 + k + 1) * D],
                                    rhs=sT[:, (t0 + k) * P : (t0 + k + 1) * P],
                                    start=(first and k == 0),
                                    stop=(last and k == t_wc - 1),
                                    skip_group_check=True,
                                )

                    for wi, w_ in enumerate(ws):
                        aggT_sb = hpool.tile([P, P], bf16, tag="aggT")
                        nc.vector.tensor_copy(out=aggT_sb[:], in_=agg_banks[wi][:])
                        # h1 reuses window wi's just-consumed PSUM bank
                        h1_ps = aggp.tile([P, D], f32, tag=f"agg{wi}", name=f"h1_ps{wi}")
                        # h1[dst, hid] = (aggT)^T @ W1
                        nc.tensor.matmul(
                            out=h1_ps[:],
                            lhsT=aggT_sb[:],
                            rhs=w1_sb[:],
                            start=True,
                            stop=True,
                        )
                        h1_sb = hpool.tile([P, D], bf16, tag="h1s")
                        nc.scalar.activation(
                            h1_sb[:], h1_ps[:], mybir.ActivationFunctionType.Relu
                        )
                        if DEBUG_DUMP:
                            dbg_a = hpool.tile([P, P], f32, tag="dbg_a")
                            nc.vector.tensor_copy(out=dbg_a[:], in_=aggT_sb[:])
                            nc.sync.dma_start(out=dbg_aggT[w_, :, :], in_=dbg_a[:])
                            dbg_h = hpool.tile([P, P], f32, tag="dbg_h")
                            nc.vector.tensor_copy(out=dbg_h[:], in_=h1_sb[:])
                            nc.sync.dma_start(out=dbg_h1[w_, :, :], in_=dbg_h[:])
                        for j in range(GT):
                            # ONE start/stop for the whole s_all bank: start=True
                            # pending-zeroes the entire 2KB zero region, so a
                            # second start (j=1) would wipe j=0's accumulation.
                            nc.tensor.matmul(
                                out=s_all[: gp[j], j * D : (j + 1) * D],
                                lhsT=kt_sb[:, wi * G + j * P : wi * G + j * P + gp[j]],
                                rhs=h1_sb[:],
                                start=(w_ == 0 and j == 0),
                                stop=(w_ == W - 1 and j == GT - 1),
                                skip_group_check=True,
                            )

                for j in range(GT):
                    s_sb = hpool.tile([P, D], f32, tag="s_sb")
                    nc.vector.tensor_copy(
                        out=s_sb[: gp[j], :], in_=s_all[: gp[j], j * D : (j + 1) * D]
                    )
                    nc.sync.dma_start(out=cc_in[j * P : j * P + gp[j], :], in_=s_sb[: gp[j], :])

        with nc.Block() as block:

            @block.gpsimd
            def _(g):
                g.collective_compute(
                    "AllReduce",
                    mybir.AluOpType.add,
                    ins=[cc_in[:]],
                    outs=[cc_out[:]],
                    replica_groups=[list(range(n_cores))],
                ).then_inc(cc_sem)
                g.wait_ge(cc_sem, rep + 1)

        with TileContext(nc) as tc:
            with (
                tc.tile_pool(name="fconst", bufs=1) as fc,
                tc.tile_pool(name="fin", bufs=2) as fin,
                tc.tile_pool(name="fpsum", bufs=2, space="PSUM") as fps,
            ):
                w2_sb = fc.tile([D, D], f32)
                nc.sync.dma_start(out=w2_sb[:], in_=w2_d[:, :])
                wo_sb = fc.tile([D, 1], f32)
                nc.sync.dma_start(out=wo_sb[:], in_=wo_d[:, :])
                rc_sb = fc.tile([P, GT], f32)
                nc.sync.dma_start(out=rc_sb[:], in_=rc_d[:, :])
                bo_sb = fc.tile([P, 1], f32)
                nc.sync.dma_start(out=bo_sb[:], in_=bo_d[:, :])
                ident2 = fc.tile([P, P], f32)
                nc.sync.dma_start(out=ident2[:], in_=id_d[:, :])

                sT_sb = fc.tile([D, G], f32)
                for j in range(GT):
                    s_in = fin.tile([gp[j], D], f32, tag="s_in")
                    # gpsimd: per-engine program order places this after the
                    # collective wait above
                    nc.gpsimd.dma_start(out=s_in[:], in_=cc_out[j * P : j * P + gp[j], :])
                    s_sc = fin.tile([gp[j], D], f32, tag="s_sc")
                    nc.vector.tensor_scalar_mul(
                        out=s_sc[:], in0=s_in[:], scalar1=rc_sb[: gp[j], j : j + 1]
                    )
                    stp = fps.tile([D, gp[j]], f32, tag="stp")
                    nc.tensor.transpose(stp[:], s_sc[:], ident2[: gp[j], : gp[j]])
                    nc.vector.tensor_copy(out=sT_sb[:, j * P : j * P + gp[j]], in_=stp[:])

                g2_ps = fps.tile([D, G], f32, tag="g2")
                nc.tensor.matmul(
                    out=g2_ps[:], lhsT=w2_sb[:], rhs=sT_sb[:], start=True, stop=True
                )
                g2_sb = fc.tile([D, G], f32)
                nc.vector.tensor_copy(out=g2_sb[:], in_=g2_ps[:])

                for j in range(GT):
                    o_ps = fps.tile([gp[j], 1], f32, tag="o_ps")
                    nc.tensor.matmul(
                        out=o_ps[:],
                        lhsT=g2_sb[:, j * P : j * P + gp[j]],
                        rhs=wo_sb[:],
                        start=True,
                        stop=True,
                    )
                    o_sb = fin.tile([gp[j], 1], f32, tag="o_sb")
                    nc.scalar.activation(
                        o_sb[:],
                        o_ps[:],
                        mybir.ActivationFunctionType.Sigmoid,
                        bias=bo_sb[: gp[j], :],
                    )
                    nc.sync.dma_start(out=out_d[j * P : j * P + gp[j], :], in_=o_sb[:])

    nc.compile()
    return nc


# --------------------------------------------------------------------------
# runners
# --------------------------------------------------------------------------
def make_in_maps(x, edge_index, batch, W1, W2, Wout, bout, n_cores, n_graphs):
    x = np.ascontiguousarray(x, np.float32)
    prep = host_prep(x, edge_index, batch, n_cores, n_graphs)
    struct = prep["struct"]
    G, GT = n_graphs, _ceil(n_graphs, P)
    recip_pad = np.ones(GT * P, np.float32)
    recip_pad[:G] = prep["recip"]
    recip_resh = np.ascontiguousarray(recip_pad.reshape(GT, P).T)
    boutb = np.full((P, 1), np.float32(np.asarray(bout).reshape(-1)[0]), np.float32)

    xh = x.astype(bfloat16)
    w1h = np.ascontiguousarray(W1, np.float32).astype(bfloat16)
    iota = np.tile(
        np.arange(P, dtype=np.float32)[None, :], (P, struct["MAXTC"])
    ).astype(bfloat16)
    ident32 = np.eye(P, dtype=np.float32)

    in_maps = []
    for i in range(n_cores):
        c = prep["cores"][i]
        in_maps.append(
            {
                "xh": xh,
                "idx_all": c["idx_all"],
                "dst_all": c["dst_all"],
                "kt_all": c["kt_all"],
                "W1h": w1h,
                "iota": iota,
                "W2": np.ascontiguousarray(W2, np.float32),
                "Wout": np.ascontiguousarray(Wout, np.float32),
                "recip": recip_resh,
                "boutb": boutb,
                "ident32": ident32,
            }
        )
    return in_maps, prep


def run(x, edge_index, batch, W1, W2, Wout, bout, n_cores, n_graphs, trace=False):
    from concourse.bass_utils import run_bass_kernel_spmd

    in_maps, prep = make_in_maps(
        x, edge_index, batch, W1, W2, Wout, bout, n_cores, n_graphs
    )
    N, D = x.shape
    nc = build_bass(N, D, n_graphs, prep["struct"], n_cores)
    res = run_bass_kernel_spmd(nc, in_maps, core_ids=list(range(n_cores)), trace=trace)
    return res


def kernel(**inputs):
    res = run(
        inputs["x"],
        inputs["edge_index"],
        inputs["batch"],
        inputs["W1"],
        inputs["W2"],
        inputs["Wout"],
        inputs["bout"],
        n_cores=FULL_CORES,
        n_graphs=FULL_G,
        trace=False,
    )
    return np.asarray(res.results[0]["out"], np.float32)

